# revision 13
# baseline (speedup 1.0000x reference)
"""MoE-LoRA layer (nn_MoELoRALayer) as a Bass/Tile kernel for 8 Trainium2 cores.

Computation (per token n):
    logits = x @ W_router.T                    # [N, 8]
    combine = renorm(top2(softmax(logits)))    # [N, 8]
    h       = x @ A_cat.T                      # [N, 128]   (8 experts x rank 16)
    hw      = h * combine_expanded             # [N, 128]
    out     = x @ W_base.T + b + 2.0 * hw @ B_cat.T

Sharding: data-parallel over tokens (1024 per core), weights replicated.
All heavy matmuls run as float32r (full-rate fp32 on the PE at N>=256),
accumulating in fp32 PSUM.

fp32r matmuls lower to an S3_LW uop with a single sync-wait slot, so the
program is arranged so every matmul depends on at most one new semaphore:
resident weights are "absorbed" onto the PE clock by tiny transposes at
program start, the router matmul precedes the A-path matmul per K-tile
(covering the xt tile's DMA), and each output tile's accumulation opens
with the LoRA up-projection (whose PSUM-slot wait coalesces with its DVE
data wait) before the W_base K-loop streams in.

Host-side layout prep (part of sharding):
    xt   [32, 128, 1024] = x_shard.T, K-tile major  (contraction d on partitions)
    wt   [4096, 4096]    = W_base.T
    at   [128, 32, 128]  = A.transpose(2,0,1) packed per K-tile (j = e*16+r)
    wrt  [128, 32, 8]    = W_router.T packed per K-tile
    bft  [128, 4096]     = 2.0 * B.transpose(0,2,1).reshape(128, 4096)
    emat [8, 128]        = expansion matrix (emat[e, j] = j//16 == e)
"""

import numpy as np

import concourse.bacc as bacc
import concourse.bass as bass
import concourse.mybir as mybir
import concourse.tile as tile
from concourse.bass_utils import run_bass_kernel_spmd

N_CORES = 8
D_IN = 4096
D_OUT = 4096
N_EXP = 8
R = 16
J = N_EXP * R           # 128
SCALING = 2.0
TOK = 1024              # tokens per core
K_TILES = D_IN // 128   # 32
N_TILES = TOK // 128    # 8
O_TILES = D_OUT // 512  # 8
BLK = 512               # token block for phase 1
N_BLKS = TOK // BLK     # 2

F32 = mybir.dt.float32
F32R = mybir.dt.float32r

_CACHE = {}


def _build_program(finalize=True):
    key = ("nc", finalize)
    if key in _CACHE:
        return _CACHE[key]

    nc = bacc.Bacc(trn_type="TRN2")

    xt_d = nc.dram_tensor("xt", [K_TILES, 128, TOK], F32R, kind="ExternalInput")
    wt_d = nc.dram_tensor("wt", [D_IN, D_OUT], F32R, kind="ExternalInput")
    at_d = nc.dram_tensor("at", [128, K_TILES, J], F32R, kind="ExternalInput")
    bft_d = nc.dram_tensor("bft", [J, D_OUT], F32R, kind="ExternalInput")
    bvec_d = nc.dram_tensor("bvec", [D_OUT], F32, kind="ExternalInput")
    pk_d = nc.dram_tensor("pk", [128, 512], F32, kind="ExternalInput")
    out_d = nc.dram_tensor("out", [TOK, D_OUT], F32, kind="ExternalOutput")

    xt = xt_d[:]
    wt = wt_d[:]
    out_ap = out_d[:]

    mm = nc.tensor.matmul

    with tile.TileContext(nc) as tc:
        with (
            tc.tile_pool(name="xt_pool", bufs=K_TILES) as xt_pool,
            tc.tile_pool(name="res", bufs=1) as res,
            tc.tile_pool(name="wt_pool", bufs=3) as wt_pool,
            tc.tile_pool(name="out_pool", bufs=3) as out_pool,
            tc.tile_pool(name="rsm", bufs=2) as rsm,
            tc.tile_pool(name="rbig", bufs=1) as rbig,
            tc.tile_pool(name="ps", bufs=8, space="PSUM") as ps,
        ):
            # ---- resident loads ----
            xts = []
            for k in range(K_TILES):
                t = xt_pool.tile([128, TOK], F32R, tag="xt", name=f"xt_{k}")
                nc.sync.dma_start(out=t, in_=xt[k])
                xts.append(t)

            at_sb = res.tile([128, K_TILES, J], F32R)
            nc.sync.dma_start(out=at_sb, in_=at_d[:])
            bft_sb = res.tile([J, D_OUT], F32R)
            nc.sync.dma_start(out=bft_sb, in_=bft_d[:])
            pkr = res.tile([128, 384], F32R)
            nc.sync.dma_start(out=pkr, in_=pk_d[:, 0:384].bitcast(F32R))
            pkf = res.tile([128, 128], F32)
            nc.sync.dma_start(out=pkf, in_=pk_d[:, 384:512])
            wrt_sb = pkr[:, 0:256].rearrange("p (k e) -> p k e", e=N_EXP)
            emat_sb = pkr[0:N_EXP, 256:384]
            ident_sb = pkf
            bias_sb = res.tile([128, D_OUT], F32)
            nc.gpsimd.dma_start(
                out=bias_sb, in_=bvec_d[:].partition_broadcast(128)
            )
            hwt_sb = res.tile([J, TOK], F32R)

            # bias arrives on a SWDGE queue; observe it on the DVE clock once.
            btch = rsm.tile([1, 1], F32, tag="btch")
            nc.vector.tensor_copy(out=btch, in_=bias_sb[0:1, 0:1])

            # ---- phase 1: router + LoRA down-projection, per 512-token block ----
            for b in range(N_BLKS):
                bsl = slice(b * BLK, (b + 1) * BLK)
                pr = ps.tile([128, 512], F32, tag="ps")   # router logits.T
                ph = ps.tile([128, 512], F32, tag="ps")   # hT = A_cat @ x.T
                if b == 0:
                    # Absorb resident-load DMA semaphores onto the PE clock
                    # (fp32r matmuls have one sync-wait slot). Dummy transposes
                    # into pr/ph, each waiting on exactly one DMA queue; the
                    # start=True accumulations below overwrite them.
                    i1 = pkf[0:1, 0:1]
                    nc.tensor.transpose(out=pr[:1, 0:1], in_=i1, identity=i1)
                    nc.tensor.transpose(
                        out=pr[:1, 1:2], in_=pkr[0:1, 0:1].bitcast(F32), identity=i1
                    )
                    nc.tensor.transpose(
                        out=ph[:1, 0:1], in_=bft_sb[0:1, 0:1].bitcast(F32),
                        identity=i1,
                    )
                for k in range(K_TILES):
                    # router first: its wait covers xts[k] for the A-path mm
                    mm(pr[:N_EXP, :], wrt_sb[:, k, :], xts[k][:, bsl],
                       start=(k == 0), stop=(k == K_TILES - 1))
                    mm(ph, at_sb[:, k, :], xts[k][:, bsl],
                       start=(k == 0), stop=(k == K_TILES - 1))

                logits_sb = rbig.tile([N_EXP, BLK], F32, tag="lg")
                nc.vector.tensor_copy(out=logits_sb, in_=pr[:N_EXP, :])
                combt_sb = rbig.tile([N_EXP, BLK], F32R, tag="ct")

                for c in range(BLK // 128):
                    csl = slice(c * 128, (c + 1) * 128)
                    # transpose [8, 128] -> [128, 8] (token-major)
                    pt = ps.tile([128, 512], F32, tag="ps")
                    nc.tensor.transpose(
                        out=pt[:, :N_EXP],
                        in_=logits_sb[:, csl],
                        identity=ident_sb[:N_EXP, :N_EXP],
                    )
                    ltok = rsm.tile([128, N_EXP], F32, tag="lt")
                    nc.vector.tensor_copy(out=ltok, in_=pt[:, :N_EXP])

                    # top-2 renormalized softmax weights, exact algebra:
                    #   m1 = max_e l; t = l - m1; m2 = max_e (t | top1 -> -inf)
                    #   combine_e = [t >= m2] * exp(t) / (1 + exp(m2))
                    m1 = rsm.tile([128, 1], F32, tag="m1")
                    nc.vector.tensor_reduce(
                        m1, ltok, axis=mybir.AxisListType.X, op=mybir.AluOpType.max
                    )
                    t = rsm.tile([128, N_EXP], F32, tag="t")
                    nc.vector.tensor_scalar(
                        out=t, in0=ltok, scalar1=m1, scalar2=None,
                        op0=mybir.AluOpType.subtract,
                    )
                    eq = rsm.tile([128, N_EXP], F32, tag="eq")
                    nc.vector.tensor_scalar(
                        out=eq, in0=t, scalar1=0.0, scalar2=None,
                        op0=mybir.AluOpType.is_ge,
                    )
                    msk = rsm.tile([128, N_EXP], F32, tag="msk")
                    nc.vector.scalar_tensor_tensor(
                        out=msk, in0=eq, scalar=-1e30, in1=t,
                        op0=mybir.AluOpType.mult, op1=mybir.AluOpType.add,
                    )
                    m2 = rsm.tile([128, 1], F32, tag="m2")
                    nc.vector.tensor_reduce(
                        m2, msk, axis=mybir.AxisListType.X, op=mybir.AluOpType.max
                    )
                    e2 = rsm.tile([128, 1], F32, tag="e2")
                    nc.scalar.activation(e2, m2, mybir.ActivationFunctionType.Exp)
                    den = rsm.tile([128, 1], F32, tag="den")
                    nc.vector.tensor_scalar_add(den, e2, 1.0)
                    rec = rsm.tile([128, 1], F32, tag="rec")
                    nc.vector.reciprocal(rec, den)
                    et = rsm.tile([128, N_EXP], F32, tag="et")
                    nc.scalar.activation(et, t, mybir.ActivationFunctionType.Exp)
                    ge = rsm.tile([128, N_EXP], F32, tag="ge")
                    nc.vector.tensor_scalar(
                        out=ge, in0=t, scalar1=m2, scalar2=None,
                        op0=mybir.AluOpType.is_ge,
                    )
                    w = rsm.tile([128, N_EXP], F32, tag="w")
                    nc.vector.tensor_tensor(
                        out=w, in0=et, in1=ge, op=mybir.AluOpType.mult
                    )
                    comb = rsm.tile([128, N_EXP], F32, tag="comb")
                    nc.vector.tensor_scalar_mul(comb, w, rec)

                    # transpose back [128, 8] -> [8, 128]
                    pc = ps.tile([128, 512], F32, tag="ps")
                    nc.tensor.transpose(
                        out=pc[:N_EXP, :128], in_=comb, identity=ident_sb
                    )
                    nc.vector.tensor_copy(out=combt_sb[:, csl], in_=pc[:N_EXP, :128])

                # expand combine across the 16 ranks of each expert:
                # combine_expT[j, n] = combT[j//16, n]  via  emat.T @ combT
                pce = ps.tile([128, 512], F32, tag="ps")
                mm(pce, emat_sb, combt_sb, start=True, stop=True)
                hsb = rbig.tile([128, BLK], F32, tag="hs")
                nc.vector.tensor_copy(out=hsb, in_=ph)
                nc.vector.tensor_tensor(
                    out=hwt_sb[:, bsl], in0=hsb, in1=pce, op=mybir.AluOpType.mult
                )

            # ---- phase 2: LoRA up-projection + base GEMM + bias ----
            for o in range(O_TILES):
                osl = slice(o * 512, (o + 1) * 512)
                accs = [
                    ps.tile([128, 512], F32, tag="ps", name=f"acc_{o}_{n}")
                    for n in range(N_TILES)
                ]
                # open each accumulator with the expert contribution: its PSUM
                # slot wait (DVE release) coalesces with the hwt DVE wait.
                for n in range(N_TILES):
                    mm(accs[n], hwt_sb[:, n * 128:(n + 1) * 128],
                       bft_sb[:, osl], start=True, stop=False)
                for k in range(K_TILES):
                    wtt = wt_pool.tile([128, 512], F32R, tag="wt",
                                       name=f"wt_{o}_{k}")
                    nc.sync.dma_start(
                        out=wtt, in_=wt[k * 128:(k + 1) * 128, osl]
                    )
                    for n in range(N_TILES):
                        mm(accs[n], xts[k][:, n * 128:(n + 1) * 128], wtt,
                           start=False, stop=(k == K_TILES - 1))
                for n in range(N_TILES):
                    osb = out_pool.tile([128, 512], F32, tag="ob",
                                        name=f"ob_{o}_{n}")
                    nc.vector.tensor_tensor(
                        out=osb, in0=accs[n], in1=bias_sb[:, osl],
                        op=mybir.AluOpType.add,
                    )
                    nc.sync.dma_start(
                        out=out_ap[n * 128:(n + 1) * 128, osl], in_=osb
                    )
                    # WAR closer: makes the DVE (not the outbound DMA queue)
                    # the releaser of this staging slot, so the next tile's
                    # bias-add needs no cross-queue DMA wait.
                    nc.vector.memset(osb[0:1, 0:1], 0.0)

    if finalize:
        nc.finalize()
    _CACHE[key] = nc
    return nc


def _prep_inputs(x, W_base, b_base, W_router, A, B):
    """Shard + lay out inputs for the 8 cores. Returns list of in_maps."""
    x_flat = np.ascontiguousarray(x, dtype=np.float32).reshape(-1, D_IN)
    wt = np.ascontiguousarray(W_base.T.astype(np.float32, copy=False))
    at = np.ascontiguousarray(
        A.astype(np.float32, copy=False)
        .transpose(2, 0, 1)
        .reshape(K_TILES, 128, J)
        .transpose(1, 0, 2)
    )
    wrt = (
        W_router.T.astype(np.float32, copy=False)
        .reshape(K_TILES, 128, N_EXP)
        .transpose(1, 0, 2)
    )
    bft = np.ascontiguousarray(
        SCALING * B.astype(np.float32, copy=False).transpose(0, 2, 1).reshape(J, D_OUT)
    )
    bvec = np.ascontiguousarray(b_base, dtype=np.float32)
    # packed small residents: [:, :256] wrt, [:8, 256:384] emat, [:, 384:] ident
    pk = np.zeros((128, 512), dtype=np.float32)
    pk[:, 0:256] = wrt.reshape(128, K_TILES * N_EXP)
    pk[0:N_EXP, 256:384] = np.repeat(np.eye(N_EXP, dtype=np.float32), R, axis=1)
    pk[:, 384:512] = np.eye(128, dtype=np.float32)

    in_maps = []
    for c in range(N_CORES):
        shard = x_flat[c * TOK:(c + 1) * TOK]          # [1024, 4096]
        xt = np.ascontiguousarray(shard.T).reshape(K_TILES, 128, TOK)
        in_maps.append({
            "xt": xt, "wt": wt, "at": at, "bft": bft, "bvec": bvec, "pk": pk,
        })
    return in_maps


def _run(in_maps, trace=False, **kw):
    nc = _build_program()
    return run_bass_kernel_spmd(
        nc, in_maps, core_ids=list(range(N_CORES)), trace=trace, **kw
    )


def kernel(x, W_base, b_base, W_router, A, B):
    orig_shape = np.asarray(x).shape
    in_maps = _prep_inputs(x, W_base, b_base, W_router, A, B)
    res = _run(in_maps)
    shards = [res.results[c]["out"] for c in range(N_CORES)]
    out = np.concatenate(shards, axis=0)
    return out.reshape(*orig_shape[:-1], D_OUT).astype(np.float32, copy=False)


# revision 16
# speedup vs baseline: 1.0327x; 1.0327x over previous
"""MoE-LoRA layer (nn_MoELoRALayer) as a Bass/Tile kernel for 8 Trainium2 cores.

Computation (per token n):
    logits = x @ W_router.T                    # [N, 8]
    combine = renorm(top2(softmax(logits)))    # [N, 8]
    h       = x @ A_cat.T                      # [N, 128]   (8 experts x rank 16)
    hw      = h * combine_expanded             # [N, 128]
    out     = x @ W_base.T + b + 2.0 * hw @ B_cat.T

Sharding: data-parallel over tokens (1024 per core), weights replicated.
All heavy matmuls run as float32r (full-rate fp32 on the PE at N>=256),
accumulating in fp32 PSUM.

fp32r matmuls lower to an S3_LW uop with a single sync-wait slot, so the
program is arranged so every matmul depends on at most one new semaphore:
resident weights are "absorbed" onto the PE clock by tiny transposes at
program start, the router matmul precedes the A-path matmul per K-tile
(covering the xt tile's DMA), and each output tile's accumulation opens
with the LoRA up-projection (whose PSUM-slot wait coalesces with its DVE
data wait) before the W_base K-loop streams in.

Host-side layout prep (part of sharding):
    xt   [32, 128, 1024] = x_shard.T, K-tile major  (contraction d on partitions)
    wt   [4096, 4096]    = W_base.T
    at   [128, 32, 128]  = A.transpose(2,0,1) packed per K-tile (j = e*16+r)
    wrt  [128, 32, 8]    = W_router.T packed per K-tile
    bft  [128, 4096]     = 2.0 * B.transpose(0,2,1).reshape(128, 4096)
    emat [8, 128]        = expansion matrix (emat[e, j] = j//16 == e)
"""

import numpy as np

import concourse.bacc as bacc
import concourse.bass as bass
import concourse.mybir as mybir
import concourse.tile as tile
from concourse.bass_utils import run_bass_kernel_spmd

N_CORES = 8
D_IN = 4096
D_OUT = 4096
N_EXP = 8
R = 16
J = N_EXP * R           # 128
SCALING = 2.0
TOK = 1024              # tokens per core
K_TILES = D_IN // 128   # 32
N_TILES = TOK // 128    # 8
O_TILES = D_OUT // 512  # 8
BLK = 512               # token block for phase 1
N_BLKS = TOK // BLK     # 2

F32 = mybir.dt.float32
F32R = mybir.dt.float32r

_CACHE = {}


def _build_program(finalize=True):
    key = ("nc", finalize)
    if key in _CACHE:
        return _CACHE[key]

    nc = bacc.Bacc(trn_type="TRN2")

    xt_d = nc.dram_tensor("xt", [K_TILES, 128, TOK], F32R, kind="ExternalInput")
    wt_d = nc.dram_tensor("wt", [D_IN, D_OUT], F32R, kind="ExternalInput")
    at_d = nc.dram_tensor("at", [128, K_TILES, J], F32R, kind="ExternalInput")
    bft_d = nc.dram_tensor("bft", [J, D_OUT], F32R, kind="ExternalInput")
    bvec_d = nc.dram_tensor("bvec", [D_OUT], F32, kind="ExternalInput")
    pk_d = nc.dram_tensor("pk", [128, 512], F32, kind="ExternalInput")
    out_d = nc.dram_tensor("out", [TOK, D_OUT], F32, kind="ExternalOutput")

    xt = xt_d[:]
    wt = wt_d[:]
    out_ap = out_d[:]

    mm = nc.tensor.matmul

    with tile.TileContext(nc) as tc:
        with (
            tc.tile_pool(name="xt_pool", bufs=K_TILES) as xt_pool,
            tc.tile_pool(name="res", bufs=1) as res,
            tc.tile_pool(name="wt_pool", bufs=4) as wt_pool,
            tc.tile_pool(name="out_pool", bufs=3) as out_pool,
            tc.tile_pool(name="rsm", bufs=2) as rsm,
            tc.tile_pool(name="rbig", bufs=2) as rbig,
            tc.tile_pool(name="ps", bufs=8, space="PSUM") as ps,
        ):
            # ---- resident loads: small tensors first so phase 1 can start
            # while the xt stream is still arriving ----
            pkr = res.tile([128, 384], F32R)
            nc.sync.dma_start(out=pkr, in_=pk_d[:, 0:384].bitcast(F32R))
            pkf = res.tile([128, 128], F32)
            nc.sync.dma_start(out=pkf, in_=pk_d[:, 384:512])
            at_sb = res.tile([128, K_TILES, J], F32R)
            nc.sync.dma_start(out=at_sb, in_=at_d[:])
            bft_sb = res.tile([J, D_OUT], F32R)
            nc.sync.dma_start(out=bft_sb, in_=bft_d[:])
            wrt_sb = pkr[:, 0:256].rearrange("p (k e) -> p k e", e=N_EXP)
            emat_sb = pkr[0:N_EXP, 256:384]
            ident_sb = pkf
            bias_sb = res.tile([128, D_OUT], F32)
            nc.gpsimd.dma_start(
                out=bias_sb, in_=bvec_d[:].partition_broadcast(128)
            )
            hwt_sb = res.tile([J, TOK], F32R)

            xts = []
            for k in range(K_TILES):
                t = xt_pool.tile([128, TOK], F32R, tag="xt", name=f"xt_{k}")
                nc.sync.dma_start(out=t, in_=xt[k])
                xts.append(t)

            # bias arrives on a SWDGE queue; observe it on the DVE clock once.
            btch = rsm.tile([1, 1], F32, tag="btch")
            nc.vector.tensor_copy(out=btch, in_=bias_sb[0:1, 0:1])

            # ---- phase 1: router + LoRA down-projection ----
            # Both blocks' K-loops run back-to-back on the PE (they consume the
            # incoming xt stream in order); the serial routing tails follow and
            # overlap with the start of phase 2.
            prs, phs = [], []
            for b in range(N_BLKS):
                bsl = slice(b * BLK, (b + 1) * BLK)
                pr = ps.tile([128, 512], F32, tag="ps", name=f"pr_{b}")
                ph = ps.tile([128, 512], F32, tag="ps", name=f"ph_{b}")
                if b == 0:
                    # Absorb resident-load DMA semaphores onto the PE clock
                    # (fp32r matmuls have one sync-wait slot). Dummy transposes
                    # into pr/ph, each waiting on exactly one DMA queue; the
                    # start=True accumulations below overwrite them.
                    i1 = pkf[0:1, 0:1]
                    nc.tensor.transpose(out=pr[:1, 0:1], in_=i1, identity=i1)
                    nc.tensor.transpose(
                        out=pr[:1, 1:2], in_=pkr[0:1, 0:1].bitcast(F32), identity=i1
                    )
                    nc.tensor.transpose(
                        out=ph[:1, 0:1], in_=bft_sb[0:1, 0:1].bitcast(F32),
                        identity=i1,
                    )
                for k in range(K_TILES):
                    # router first: its wait covers xts[k] for the A-path mm
                    mm(pr[:N_EXP, :], wrt_sb[:, k, :], xts[k][:, bsl],
                       start=(k == 0), stop=(k == K_TILES - 1))
                    mm(ph, at_sb[:, k, :], xts[k][:, bsl],
                       start=(k == 0), stop=(k == K_TILES - 1))
                prs.append(pr)
                phs.append(ph)

            for b in range(N_BLKS):
                bsl = slice(b * BLK, (b + 1) * BLK)
                pr, ph = prs[b], phs[b]
                logits_sb = rbig.tile([N_EXP, BLK], F32, tag="lg", name=f"lg_{b}")
                nc.vector.tensor_copy(out=logits_sb, in_=pr[:N_EXP, :])
                combt_sb = rbig.tile([N_EXP, BLK], F32R, tag="ct", name=f"ct_{b}")

                for c in range(BLK // 128):
                    csl = slice(c * 128, (c + 1) * 128)
                    # transpose [8, 128] -> [128, 8] (token-major)
                    pt = ps.tile([128, 512], F32, tag="ps", name=f"pt_{b}_{c}")
                    nc.tensor.transpose(
                        out=pt[:, :N_EXP],
                        in_=logits_sb[:, csl],
                        identity=ident_sb[:N_EXP, :N_EXP],
                    )
                    ltok = rsm.tile([128, N_EXP], F32, tag="lt", name=f"lt_{b}_{c}")
                    nc.vector.tensor_copy(out=ltok, in_=pt[:, :N_EXP])

                    # top-2 renormalized softmax weights, exact algebra:
                    #   m1 = max_e l; t = l - m1; m2 = max_e (t | top1 -> -inf)
                    #   combine_e = [t >= m2] * exp(t) / (1 + exp(m2))
                    m1 = rsm.tile([128, 1], F32, tag="m1")
                    nc.vector.tensor_reduce(
                        m1, ltok, axis=mybir.AxisListType.X, op=mybir.AluOpType.max
                    )
                    t = rsm.tile([128, N_EXP], F32, tag="t")
                    nc.vector.tensor_scalar(
                        out=t, in0=ltok, scalar1=m1, scalar2=None,
                        op0=mybir.AluOpType.subtract,
                    )
                    eq = rsm.tile([128, N_EXP], F32, tag="eq")
                    nc.vector.tensor_scalar(
                        out=eq, in0=t, scalar1=0.0, scalar2=None,
                        op0=mybir.AluOpType.is_ge,
                    )
                    msk = rsm.tile([128, N_EXP], F32, tag="msk")
                    nc.vector.scalar_tensor_tensor(
                        out=msk, in0=eq, scalar=-1e30, in1=t,
                        op0=mybir.AluOpType.mult, op1=mybir.AluOpType.add,
                    )
                    m2 = rsm.tile([128, 1], F32, tag="m2")
                    nc.vector.tensor_reduce(
                        m2, msk, axis=mybir.AxisListType.X, op=mybir.AluOpType.max
                    )
                    e2 = rsm.tile([128, 1], F32, tag="e2")
                    nc.scalar.activation(e2, m2, mybir.ActivationFunctionType.Exp)
                    den = rsm.tile([128, 1], F32, tag="den")
                    nc.vector.tensor_scalar_add(den, e2, 1.0)
                    rec = rsm.tile([128, 1], F32, tag="rec")
                    nc.vector.reciprocal(rec, den)
                    et = rsm.tile([128, N_EXP], F32, tag="et")
                    nc.scalar.activation(et, t, mybir.ActivationFunctionType.Exp)
                    ge = rsm.tile([128, N_EXP], F32, tag="ge")
                    nc.vector.tensor_scalar(
                        out=ge, in0=t, scalar1=m2, scalar2=None,
                        op0=mybir.AluOpType.is_ge,
                    )
                    w = rsm.tile([128, N_EXP], F32, tag="w")
                    nc.vector.tensor_tensor(
                        out=w, in0=et, in1=ge, op=mybir.AluOpType.mult
                    )
                    comb = rsm.tile([128, N_EXP], F32, tag="comb")
                    nc.vector.tensor_scalar_mul(comb, w, rec)

                    # transpose back [128, 8] -> [8, 128]
                    pc = ps.tile([128, 512], F32, tag="ps", name=f"pc_{b}_{c}")
                    nc.tensor.transpose(
                        out=pc[:N_EXP, :128], in_=comb, identity=ident_sb
                    )
                    nc.vector.tensor_copy(out=combt_sb[:, csl], in_=pc[:N_EXP, :128])

                # expand combine across the 16 ranks of each expert:
                # combine_expT[j, n] = combT[j//16, n]  via  emat.T @ combT
                pce = ps.tile([128, 512], F32, tag="ps", name=f"pce_{b}")
                mm(pce, emat_sb, combt_sb, start=True, stop=True)
                hsb = rbig.tile([128, BLK], F32, tag="hs", name=f"hs_{b}", bufs=1)
                nc.vector.tensor_copy(out=hsb, in_=ph)
                nc.vector.tensor_tensor(
                    out=hwt_sb[:, bsl], in0=hsb, in1=pce, op=mybir.AluOpType.mult
                )

            # ---- phase 2: LoRA up-projection + base GEMM + bias ----
            for o in range(O_TILES):
                osl = slice(o * 512, (o + 1) * 512)
                accs = [
                    ps.tile([128, 512], F32, tag="ps", name=f"acc_{o}_{n}")
                    for n in range(N_TILES)
                ]
                # open each accumulator with the expert contribution: its PSUM
                # slot wait (DVE release) coalesces with the hwt DVE wait.
                for n in range(N_TILES):
                    mm(accs[n], hwt_sb[:, n * 128:(n + 1) * 128],
                       bft_sb[:, osl], start=True, stop=False)
                for k in range(K_TILES):
                    wtt = wt_pool.tile([128, 512], F32R, tag="wt",
                                       name=f"wt_{o}_{k}")
                    nc.scalar.dma_start(
                        out=wtt, in_=wt[k * 128:(k + 1) * 128, osl]
                    )
                    for n in range(N_TILES):
                        mm(accs[n], xts[k][:, n * 128:(n + 1) * 128], wtt,
                           start=False, stop=(k == K_TILES - 1))
                for n in range(N_TILES):
                    osb = out_pool.tile([128, 512], F32, tag="ob",
                                        name=f"ob_{o}_{n}")
                    nc.vector.tensor_tensor(
                        out=osb, in0=accs[n], in1=bias_sb[:, osl],
                        op=mybir.AluOpType.add,
                    )
                    nc.sync.dma_start(
                        out=out_ap[n * 128:(n + 1) * 128, osl], in_=osb
                    )
                    # WAR closer: makes the DVE (not the outbound DMA queue)
                    # the releaser of this staging slot, so the next tile's
                    # bias-add needs no cross-queue DMA wait.
                    nc.vector.memset(osb[0:1, 0:1], 0.0)

    if finalize:
        nc.finalize()
    _CACHE[key] = nc
    return nc


def _prep_inputs(x, W_base, b_base, W_router, A, B):
    """Shard + lay out inputs for the 8 cores. Returns list of in_maps."""
    x_flat = np.ascontiguousarray(x, dtype=np.float32).reshape(-1, D_IN)
    wt = np.ascontiguousarray(W_base.T.astype(np.float32, copy=False))
    at = np.ascontiguousarray(
        A.astype(np.float32, copy=False)
        .transpose(2, 0, 1)
        .reshape(K_TILES, 128, J)
        .transpose(1, 0, 2)
    )
    wrt = (
        W_router.T.astype(np.float32, copy=False)
        .reshape(K_TILES, 128, N_EXP)
        .transpose(1, 0, 2)
    )
    bft = np.ascontiguousarray(
        SCALING * B.astype(np.float32, copy=False).transpose(0, 2, 1).reshape(J, D_OUT)
    )
    bvec = np.ascontiguousarray(b_base, dtype=np.float32)
    # packed small residents: [:, :256] wrt, [:8, 256:384] emat, [:, 384:] ident
    pk = np.zeros((128, 512), dtype=np.float32)
    pk[:, 0:256] = wrt.reshape(128, K_TILES * N_EXP)
    pk[0:N_EXP, 256:384] = np.repeat(np.eye(N_EXP, dtype=np.float32), R, axis=1)
    pk[:, 384:512] = np.eye(128, dtype=np.float32)

    in_maps = []
    for c in range(N_CORES):
        shard = x_flat[c * TOK:(c + 1) * TOK]          # [1024, 4096]
        xt = np.ascontiguousarray(shard.T).reshape(K_TILES, 128, TOK)
        in_maps.append({
            "xt": xt, "wt": wt, "at": at, "bft": bft, "bvec": bvec, "pk": pk,
        })
    return in_maps


def _run(in_maps, trace=False, **kw):
    nc = _build_program()
    return run_bass_kernel_spmd(
        nc, in_maps, core_ids=list(range(N_CORES)), trace=trace, **kw
    )


def kernel(x, W_base, b_base, W_router, A, B):
    orig_shape = np.asarray(x).shape
    in_maps = _prep_inputs(x, W_base, b_base, W_router, A, B)
    res = _run(in_maps)
    shards = [res.results[c]["out"] for c in range(N_CORES)]
    out = np.concatenate(shards, axis=0)
    return out.reshape(*orig_shape[:-1], D_OUT).astype(np.float32, copy=False)


# revision 18
# speedup vs baseline: 1.3669x; 1.3236x over previous
"""MoE-LoRA layer (nn_MoELoRALayer) as a Bass/Tile kernel for 8 Trainium2 cores.

Computation (per token n):
    logits = x @ W_router.T                    # [N, 8]
    combine = renorm(top2(softmax(logits)))    # [N, 8]
    h       = x @ A_cat.T                      # [N, 128]   (8 experts x rank 16)
    hw      = h * combine_expanded             # [N, 128]
    out     = x @ W_base.T + b + 2.0 * hw @ B_cat.T

Sharding: data-parallel over tokens (1024 per core), weights replicated.
All heavy matmuls run as float32r (full-rate fp32 on the PE at N>=256),
accumulating in fp32 PSUM.

fp32r matmuls lower to an S3_LW uop with a single sync-wait slot, so the
program is arranged so every matmul depends on at most one new semaphore:
resident weights are "absorbed" onto the PE clock by tiny transposes at
program start, the router matmul precedes the A-path matmul per K-tile
(covering the xt tile's DMA), and each output tile's accumulation opens
with the LoRA up-projection (whose PSUM-slot wait coalesces with its DVE
data wait) before the W_base K-loop streams in.

Host-side layout prep (part of sharding):
    xt   [32, 128, 1024] = x_shard.T, K-tile major  (contraction d on partitions)
    wt   [4096, 4096]    = W_base.T
    at   [128, 32, 128]  = A.transpose(2,0,1) packed per K-tile (j = e*16+r)
    wrt  [128, 32, 8]    = W_router.T packed per K-tile
    bft  [128, 4096]     = 2.0 * B.transpose(0,2,1).reshape(128, 4096)
    emat [8, 128]        = expansion matrix (emat[e, j] = j//16 == e)
"""

import numpy as np

import concourse.bacc as bacc
import concourse.bass as bass
import concourse.mybir as mybir
import concourse.tile as tile
from concourse.bass_utils import run_bass_kernel_spmd

N_CORES = 8
D_IN = 4096
D_OUT = 4096
N_EXP = 8
R = 16
J = N_EXP * R           # 128
SCALING = 2.0
TOK = 1024              # tokens per core
K_TILES = D_IN // 128   # 32
N_TILES = TOK // 128    # 8
O_TILES = D_OUT // 512  # 8
BLK = 512               # token block for phase 1
N_BLKS = TOK // BLK     # 2

F32 = mybir.dt.float32
BF16 = mybir.dt.bfloat16

_CACHE = {}


def _build_program(finalize=True):
    key = ("nc", finalize)
    if key in _CACHE:
        return _CACHE[key]

    nc = bacc.Bacc(trn_type="TRN2")

    xt_d = nc.dram_tensor("xt", [K_TILES, 128, TOK], BF16, kind="ExternalInput")
    wt_d = nc.dram_tensor("wt", [D_IN, D_OUT], BF16, kind="ExternalInput")
    at_d = nc.dram_tensor("at", [128, K_TILES, J], BF16, kind="ExternalInput")
    bft_d = nc.dram_tensor("bft", [J, D_OUT], BF16, kind="ExternalInput")
    bvec_d = nc.dram_tensor("bvec", [D_OUT], F32, kind="ExternalInput")
    pkb_d = nc.dram_tensor("pkb", [128, 384], BF16, kind="ExternalInput")
    id_d = nc.dram_tensor("ident", [128, 128], F32, kind="ExternalInput")
    out_d = nc.dram_tensor("out", [TOK, D_OUT], F32, kind="ExternalOutput")

    xt = xt_d[:]
    wt = wt_d[:]
    out_ap = out_d[:]

    mm = nc.tensor.matmul

    with tile.TileContext(nc) as tc:
        with (
            tc.tile_pool(name="xt_pool", bufs=K_TILES) as xt_pool,
            tc.tile_pool(name="res", bufs=1) as res,
            tc.tile_pool(name="wt_pool", bufs=8) as wt_pool,
            tc.tile_pool(name="out_pool", bufs=6) as out_pool,
            tc.tile_pool(name="rsm", bufs=2) as rsm,
            tc.tile_pool(name="rbig", bufs=2) as rbig,
            tc.tile_pool(name="ps", bufs=8, space="PSUM") as ps,
        ):
            # ---- resident loads: small tensors first so phase 1 can start
            # while the xt stream is still arriving ----
            pkr = res.tile([128, 384], BF16)
            nc.sync.dma_start(out=pkr, in_=pkb_d[:])
            pkf = res.tile([128, 128], F32)
            nc.sync.dma_start(out=pkf, in_=id_d[:])
            at_sb = res.tile([128, K_TILES, J], BF16)
            nc.sync.dma_start(out=at_sb, in_=at_d[:])
            bft_sb = res.tile([J, D_OUT], BF16)
            nc.sync.dma_start(out=bft_sb, in_=bft_d[:])
            wrt_sb = pkr[:, 0:256].rearrange("p (k e) -> p k e", e=N_EXP)
            emat_sb = pkr[0:N_EXP, 256:384]
            ident_sb = pkf
            bias_sb = res.tile([128, D_OUT], F32)
            nc.gpsimd.dma_start(
                out=bias_sb, in_=bvec_d[:].partition_broadcast(128)
            )
            hwt_sb = res.tile([J, TOK], BF16)

            xts = []
            for k in range(K_TILES):
                t = xt_pool.tile([128, TOK], BF16, tag="xt", name=f"xt_{k}")
                nc.sync.dma_start(out=t, in_=xt[k])
                xts.append(t)

            # bias arrives on a SWDGE queue; observe it on the DVE clock once.
            btch = rsm.tile([1, 1], F32, tag="btch")
            nc.vector.tensor_copy(out=btch, in_=bias_sb[0:1, 0:1])

            # ---- phase 1: router + LoRA down-projection ----
            # Both blocks' K-loops run back-to-back on the PE (they consume the
            # incoming xt stream in order); the serial routing tails follow and
            # overlap with the start of phase 2.
            prs, phs = [], []
            for b in range(N_BLKS):
                bsl = slice(b * BLK, (b + 1) * BLK)
                pr = ps.tile([128, 512], F32, tag="ps", name=f"pr_{b}")
                ph = ps.tile([128, 512], F32, tag="ps", name=f"ph_{b}")
                if b == 0:
                    # Absorb resident-load DMA semaphores onto the PE clock
                    # (fp32r matmuls have one sync-wait slot). Dummy transposes
                    # into pr/ph, each waiting on exactly one DMA queue; the
                    # start=True accumulations below overwrite them.
                    i1 = pkf[0:1, 0:1]
                    b1 = pkr[0:1, 0:1]
                    nc.tensor.transpose(out=pr[:1, 0:1], in_=i1, identity=i1)
                    nc.tensor.transpose(
                        out=pr[:1, 1:2].bitcast(BF16)[:, 0:1], in_=b1, identity=b1
                    )
                    nc.tensor.transpose(
                        out=ph[:1, 0:1].bitcast(BF16)[:, 0:1],
                        in_=bft_sb[0:1, 0:1], identity=b1,
                    )
                for k in range(K_TILES):
                    # router first: its wait covers xts[k] for the A-path mm
                    mm(pr[:N_EXP, :], wrt_sb[:, k, :], xts[k][:, bsl],
                       start=(k == 0), stop=(k == K_TILES - 1))
                    mm(ph, at_sb[:, k, :], xts[k][:, bsl],
                       start=(k == 0), stop=(k == K_TILES - 1))
                prs.append(pr)
                phs.append(ph)

            for b in range(N_BLKS):
                bsl = slice(b * BLK, (b + 1) * BLK)
                pr, ph = prs[b], phs[b]
                logits_sb = rbig.tile([N_EXP, BLK], F32, tag="lg", name=f"lg_{b}")
                nc.vector.tensor_copy(out=logits_sb, in_=pr[:N_EXP, :])
                combt_sb = rbig.tile([N_EXP, BLK], BF16, tag="ct", name=f"ct_{b}")

                for c in range(BLK // 128):
                    csl = slice(c * 128, (c + 1) * 128)
                    # transpose [8, 128] -> [128, 8] (token-major)
                    pt = ps.tile([128, 512], F32, tag="ps", name=f"pt_{b}_{c}")
                    nc.tensor.transpose(
                        out=pt[:, :N_EXP],
                        in_=logits_sb[:, csl],
                        identity=ident_sb[:N_EXP, :N_EXP],
                    )
                    ltok = rsm.tile([128, N_EXP], F32, tag="lt", name=f"lt_{b}_{c}")
                    nc.vector.tensor_copy(out=ltok, in_=pt[:, :N_EXP])

                    # top-2 renormalized softmax weights, exact algebra:
                    #   m1 = max_e l; t = l - m1; m2 = max_e (t | top1 -> -inf)
                    #   combine_e = [t >= m2] * exp(t) / (1 + exp(m2))
                    m1 = rsm.tile([128, 1], F32, tag="m1")
                    nc.vector.tensor_reduce(
                        m1, ltok, axis=mybir.AxisListType.X, op=mybir.AluOpType.max
                    )
                    t = rsm.tile([128, N_EXP], F32, tag="t")
                    nc.vector.tensor_scalar(
                        out=t, in0=ltok, scalar1=m1, scalar2=None,
                        op0=mybir.AluOpType.subtract,
                    )
                    eq = rsm.tile([128, N_EXP], F32, tag="eq")
                    nc.vector.tensor_scalar(
                        out=eq, in0=t, scalar1=0.0, scalar2=None,
                        op0=mybir.AluOpType.is_ge,
                    )
                    msk = rsm.tile([128, N_EXP], F32, tag="msk")
                    nc.vector.scalar_tensor_tensor(
                        out=msk, in0=eq, scalar=-1e30, in1=t,
                        op0=mybir.AluOpType.mult, op1=mybir.AluOpType.add,
                    )
                    m2 = rsm.tile([128, 1], F32, tag="m2")
                    nc.vector.tensor_reduce(
                        m2, msk, axis=mybir.AxisListType.X, op=mybir.AluOpType.max
                    )
                    e2 = rsm.tile([128, 1], F32, tag="e2")
                    nc.scalar.activation(e2, m2, mybir.ActivationFunctionType.Exp)
                    den = rsm.tile([128, 1], F32, tag="den")
                    nc.vector.tensor_scalar_add(den, e2, 1.0)
                    rec = rsm.tile([128, 1], F32, tag="rec")
                    nc.vector.reciprocal(rec, den)
                    et = rsm.tile([128, N_EXP], F32, tag="et")
                    nc.scalar.activation(et, t, mybir.ActivationFunctionType.Exp)
                    ge = rsm.tile([128, N_EXP], F32, tag="ge")
                    nc.vector.tensor_scalar(
                        out=ge, in0=t, scalar1=m2, scalar2=None,
                        op0=mybir.AluOpType.is_ge,
                    )
                    w = rsm.tile([128, N_EXP], F32, tag="w")
                    nc.vector.tensor_tensor(
                        out=w, in0=et, in1=ge, op=mybir.AluOpType.mult
                    )
                    comb = rsm.tile([128, N_EXP], F32, tag="comb")
                    nc.vector.tensor_scalar_mul(comb, w, rec)

                    # transpose back [128, 8] -> [8, 128]
                    pc = ps.tile([128, 512], F32, tag="ps", name=f"pc_{b}_{c}")
                    nc.tensor.transpose(
                        out=pc[:N_EXP, :128], in_=comb, identity=ident_sb
                    )
                    nc.vector.tensor_copy(out=combt_sb[:, csl], in_=pc[:N_EXP, :128])

                # expand combine across the 16 ranks of each expert:
                # combine_expT[j, n] = combT[j//16, n]  via  emat.T @ combT
                pce = ps.tile([128, 512], F32, tag="ps", name=f"pce_{b}")
                mm(pce, emat_sb, combt_sb, start=True, stop=True)
                hsb = rbig.tile([128, BLK], F32, tag="hs", name=f"hs_{b}")
                nc.vector.tensor_copy(out=hsb, in_=ph)
                nc.vector.tensor_tensor(
                    out=hwt_sb[:, bsl], in0=hsb, in1=pce, op=mybir.AluOpType.mult
                )

            # ---- phase 2: LoRA up-projection + base GEMM + bias ----
            for o in range(O_TILES):
                osl = slice(o * 512, (o + 1) * 512)
                accs = [
                    ps.tile([128, 512], F32, tag="ps", name=f"acc_{o}_{n}")
                    for n in range(N_TILES)
                ]
                # open each accumulator with the expert contribution: its PSUM
                # slot wait (DVE release) coalesces with the hwt DVE wait.
                for n in range(N_TILES):
                    mm(accs[n], hwt_sb[:, n * 128:(n + 1) * 128],
                       bft_sb[:, osl], start=True, stop=False)
                for k in range(K_TILES):
                    wtt = wt_pool.tile([128, 512], BF16, tag="wt",
                                       name=f"wt_{o}_{k}")
                    nc.scalar.dma_start(
                        out=wtt, in_=wt[k * 128:(k + 1) * 128, osl]
                    )
                    for n in range(N_TILES):
                        mm(accs[n], xts[k][:, n * 128:(n + 1) * 128], wtt,
                           start=False, stop=(k == K_TILES - 1))
                for n in range(N_TILES):
                    osb = out_pool.tile([128, 512], F32, tag="ob",
                                        name=f"ob_{o}_{n}")
                    nc.vector.tensor_tensor(
                        out=osb, in0=accs[n], in1=bias_sb[:, osl],
                        op=mybir.AluOpType.add,
                    )
                    nc.sync.dma_start(
                        out=out_ap[n * 128:(n + 1) * 128, osl], in_=osb
                    )
                    # WAR closer: makes the DVE (not the outbound DMA queue)
                    # the releaser of this staging slot, so the next tile's
                    # bias-add needs no cross-queue DMA wait.
                    nc.vector.memset(osb[0:1, 0:1], 0.0)

    if finalize:
        nc.finalize()
    _CACHE[key] = nc
    return nc


def _prep_inputs(x, W_base, b_base, W_router, A, B):
    """Shard + lay out inputs for the 8 cores. Returns list of in_maps."""
    import ml_dtypes
    bf16 = ml_dtypes.bfloat16
    x_flat = np.ascontiguousarray(x, dtype=np.float32).reshape(-1, D_IN)
    wt = np.ascontiguousarray(W_base.T.astype(bf16))
    at = np.ascontiguousarray(
        A.astype(np.float32, copy=False)
        .transpose(2, 0, 1)
        .reshape(K_TILES, 128, J)
        .transpose(1, 0, 2)
        .astype(bf16)
    )
    wrt = (
        W_router.T.astype(np.float32, copy=False)
        .reshape(K_TILES, 128, N_EXP)
        .transpose(1, 0, 2)
    )
    bft = np.ascontiguousarray(
        (SCALING * B.astype(np.float32, copy=False).transpose(0, 2, 1)
         .reshape(J, D_OUT)).astype(bf16)
    )
    bvec = np.ascontiguousarray(b_base, dtype=np.float32)
    # packed bf16 residents: [:, :256] wrt, [:8, 256:384] emat
    pkb = np.zeros((128, 384), dtype=bf16)
    pkb[:, 0:256] = wrt.reshape(128, K_TILES * N_EXP).astype(bf16)
    pkb[0:N_EXP, 256:384] = np.repeat(
        np.eye(N_EXP, dtype=np.float32), R, axis=1
    ).astype(bf16)
    ident = np.eye(128, dtype=np.float32)

    in_maps = []
    for c in range(N_CORES):
        shard = x_flat[c * TOK:(c + 1) * TOK]          # [1024, 4096]
        xt = np.ascontiguousarray(shard.T.astype(bf16)).reshape(K_TILES, 128, TOK)
        in_maps.append({
            "xt": xt, "wt": wt, "at": at, "bft": bft, "bvec": bvec,
            "pkb": pkb, "ident": ident,
        })
    return in_maps


def _run(in_maps, trace=False, **kw):
    nc = _build_program()
    return run_bass_kernel_spmd(
        nc, in_maps, core_ids=list(range(N_CORES)), trace=trace, **kw
    )


def kernel(x, W_base, b_base, W_router, A, B):
    orig_shape = np.asarray(x).shape
    in_maps = _prep_inputs(x, W_base, b_base, W_router, A, B)
    res = _run(in_maps)
    shards = [res.results[c]["out"] for c in range(N_CORES)]
    out = np.concatenate(shards, axis=0)
    return out.reshape(*orig_shape[:-1], D_OUT).astype(np.float32, copy=False)


# revision 19
# speedup vs baseline: 1.3835x; 1.0121x over previous
"""MoE-LoRA layer (nn_MoELoRALayer) as a Bass/Tile kernel for 8 Trainium2 cores.

Computation (per token n):
    logits = x @ W_router.T                    # [N, 8]
    combine = renorm(top2(softmax(logits)))    # [N, 8]
    h       = x @ A_cat.T                      # [N, 128]   (8 experts x rank 16)
    hw      = h * combine_expanded             # [N, 128]
    out     = x @ W_base.T + b + 2.0 * hw @ B_cat.T

Sharding: data-parallel over tokens (1024 per core), weights replicated.
All heavy matmuls run as float32r (full-rate fp32 on the PE at N>=256),
accumulating in fp32 PSUM.

fp32r matmuls lower to an S3_LW uop with a single sync-wait slot, so the
program is arranged so every matmul depends on at most one new semaphore:
resident weights are "absorbed" onto the PE clock by tiny transposes at
program start, the router matmul precedes the A-path matmul per K-tile
(covering the xt tile's DMA), and each output tile's accumulation opens
with the LoRA up-projection (whose PSUM-slot wait coalesces with its DVE
data wait) before the W_base K-loop streams in.

Host-side layout prep (part of sharding):
    xt   [32, 128, 1024] = x_shard.T, K-tile major  (contraction d on partitions)
    wt   [4096, 4096]    = W_base.T
    at   [128, 32, 128]  = A.transpose(2,0,1) packed per K-tile (j = e*16+r)
    wrt  [128, 32, 8]    = W_router.T packed per K-tile
    bft  [128, 4096]     = 2.0 * B.transpose(0,2,1).reshape(128, 4096)
    emat [8, 128]        = expansion matrix (emat[e, j] = j//16 == e)
"""

import numpy as np

import concourse.bacc as bacc
import concourse.bass as bass
import concourse.mybir as mybir
import concourse.tile as tile
from concourse.bass_utils import run_bass_kernel_spmd

N_CORES = 8
D_IN = 4096
D_OUT = 4096
N_EXP = 8
R = 16
J = N_EXP * R           # 128
SCALING = 2.0
TOK = 1024              # tokens per core
K_TILES = D_IN // 128   # 32
N_TILES = TOK // 128    # 8
O_TILES = D_OUT // 512  # 8
BLK = 512               # token block for phase 1
N_BLKS = TOK // BLK     # 2

F32 = mybir.dt.float32
BF16 = mybir.dt.bfloat16

_CACHE = {}


def _build_program(finalize=True):
    key = ("nc", finalize)
    if key in _CACHE:
        return _CACHE[key]

    nc = bacc.Bacc(trn_type="TRN2")

    xt_d = nc.dram_tensor("xt", [K_TILES, 128, TOK], BF16, kind="ExternalInput")
    wt_d = nc.dram_tensor("wt", [D_IN, D_OUT], BF16, kind="ExternalInput")
    at_d = nc.dram_tensor("at", [128, K_TILES, J], BF16, kind="ExternalInput")
    bft_d = nc.dram_tensor("bft", [J, D_OUT], BF16, kind="ExternalInput")
    bvec_d = nc.dram_tensor("bvec", [D_OUT], F32, kind="ExternalInput")
    pkb_d = nc.dram_tensor("pkb", [128, 384], BF16, kind="ExternalInput")
    id_d = nc.dram_tensor("ident", [128, 128], F32, kind="ExternalInput")
    out_d = nc.dram_tensor("out", [TOK, D_OUT], F32, kind="ExternalOutput")

    xt = xt_d[:]
    wt = wt_d[:]
    out_ap = out_d[:]

    mm = nc.tensor.matmul

    with tile.TileContext(nc) as tc:
        with (
            tc.tile_pool(name="xt_pool", bufs=K_TILES) as xt_pool,
            tc.tile_pool(name="res", bufs=1) as res,
            tc.tile_pool(name="wt_pool", bufs=8) as wt_pool,
            tc.tile_pool(name="out_pool", bufs=6) as out_pool,
            tc.tile_pool(name="rsm", bufs=2) as rsm,
            tc.tile_pool(name="rbig", bufs=2) as rbig,
            tc.tile_pool(name="ps", bufs=8, space="PSUM") as ps,
        ):
            # ---- resident loads: small tensors first so phase 1 can start
            # while the xt stream is still arriving ----
            pkr = res.tile([128, 384], BF16)
            nc.sync.dma_start(out=pkr, in_=pkb_d[:])
            pkf = res.tile([128, 128], F32)
            nc.sync.dma_start(out=pkf, in_=id_d[:])
            at_sb = res.tile([128, K_TILES, J], BF16)
            nc.sync.dma_start(out=at_sb, in_=at_d[:])
            bft_sb = res.tile([J, D_OUT], BF16)
            nc.sync.dma_start(out=bft_sb, in_=bft_d[:])
            wrt_sb = pkr[:, 0:256].rearrange("p (k e) -> p k e", e=N_EXP)
            emat_sb = pkr[0:N_EXP, 256:384]
            ident_sb = pkf
            bias_sb = res.tile([128, D_OUT], F32)
            nc.gpsimd.dma_start(
                out=bias_sb, in_=bvec_d[:].partition_broadcast(128)
            )
            hwt_sb = res.tile([J, TOK], BF16)

            xts = []
            for k in range(K_TILES):
                t = xt_pool.tile([128, TOK], BF16, tag="xt", name=f"xt_{k}")
                nc.sync.dma_start(out=t, in_=xt[k])
                xts.append(t)

            # bias arrives on a SWDGE queue; observe it on the DVE clock once.
            btch = rsm.tile([1, 1], F32, tag="btch")
            nc.vector.tensor_copy(out=btch, in_=bias_sb[0:1, 0:1])

            # ---- phase 1: router + LoRA down-projection ----
            # Both blocks' K-loops run back-to-back on the PE (they consume the
            # incoming xt stream in order); the serial routing tails follow and
            # overlap with the start of phase 2.
            prs, phs = [], []
            for b in range(N_BLKS):
                bsl = slice(b * BLK, (b + 1) * BLK)
                pr = ps.tile([128, 512], F32, tag="ps", name=f"pr_{b}")
                ph = ps.tile([128, 512], F32, tag="ps", name=f"ph_{b}")
                if b == 0:
                    # Absorb resident-load DMA semaphores onto the PE clock
                    # (fp32r matmuls have one sync-wait slot). Dummy transposes
                    # into pr/ph, each waiting on exactly one DMA queue; the
                    # start=True accumulations below overwrite them.
                    i1 = pkf[0:1, 0:1]
                    b1 = pkr[0:1, 256:257]  # emat[0,0] == 1.0
                    nc.tensor.transpose(out=pr[:1, 0:1], in_=i1, identity=i1)
                    nc.tensor.transpose(
                        out=pr[:1, 1:2].bitcast(BF16)[:, 0:1], in_=b1, identity=b1
                    )
                    nc.tensor.transpose(
                        out=ph[:1, 0:1].bitcast(BF16)[:, 0:1],
                        in_=bft_sb[0:1, 0:1], identity=b1,
                    )
                for k in range(K_TILES):
                    # router first: its wait covers xts[k] for the A-path mm
                    mm(pr[:N_EXP, :], wrt_sb[:, k, :], xts[k][:, bsl],
                       start=(k == 0), stop=(k == K_TILES - 1))
                    mm(ph, at_sb[:, k, :], xts[k][:, bsl],
                       start=(k == 0), stop=(k == K_TILES - 1))
                prs.append(pr)
                phs.append(ph)

            logits = []
            for b in range(N_BLKS):
                logits_sb = rbig.tile([N_EXP, BLK], F32, tag="lg", name=f"lg_{b}")
                nc.vector.tensor_copy(out=logits_sb, in_=prs[b][:N_EXP, :])
                logits.append(logits_sb)

            # all forward transposes first: the PE never queues behind the
            # serial DVE/ACT routing chain (FIFO head-of-line blocking)
            ltoks = {}
            for b in range(N_BLKS):
                for c in range(BLK // 128):
                    csl = slice(c * 128, (c + 1) * 128)
                    pt = ps.tile([128, 512], F32, tag="ps", name=f"pt_{b}_{c}")
                    nc.tensor.transpose(
                        out=pt[:, :N_EXP],
                        in_=logits[b][:, csl],
                        identity=ident_sb[:N_EXP, :N_EXP],
                    )
                    ltok = rsm.tile([128, N_EXP], F32, tag="lt",
                                    name=f"lt_{b}_{c}", bufs=8)
                    nc.vector.tensor_copy(out=ltok, in_=pt[:, :N_EXP])
                    ltoks[b, c] = ltok

            # top-2 renormalized softmax weights, exact algebra:
            #   m1 = max_e l; t = l - m1; m2 = max_e (t | top1 -> -inf)
            #   combine_e = [t >= m2] * exp(t) / (1 + exp(m2))
            combs = {}
            for b in range(N_BLKS):
                for c in range(BLK // 128):
                    ltok = ltoks[b, c]
                    m1 = rsm.tile([128, 1], F32, tag="m1")
                    nc.vector.tensor_reduce(
                        m1, ltok, axis=mybir.AxisListType.X, op=mybir.AluOpType.max
                    )
                    t = rsm.tile([128, N_EXP], F32, tag="t")
                    nc.vector.tensor_scalar(
                        out=t, in0=ltok, scalar1=m1, scalar2=None,
                        op0=mybir.AluOpType.subtract,
                    )
                    eq = rsm.tile([128, N_EXP], F32, tag="eq")
                    nc.vector.tensor_scalar(
                        out=eq, in0=t, scalar1=0.0, scalar2=None,
                        op0=mybir.AluOpType.is_ge,
                    )
                    msk = rsm.tile([128, N_EXP], F32, tag="msk")
                    nc.vector.scalar_tensor_tensor(
                        out=msk, in0=eq, scalar=-1e30, in1=t,
                        op0=mybir.AluOpType.mult, op1=mybir.AluOpType.add,
                    )
                    m2 = rsm.tile([128, 1], F32, tag="m2")
                    nc.vector.tensor_reduce(
                        m2, msk, axis=mybir.AxisListType.X, op=mybir.AluOpType.max
                    )
                    e2 = rsm.tile([128, 1], F32, tag="e2")
                    nc.scalar.activation(e2, m2, mybir.ActivationFunctionType.Exp)
                    den = rsm.tile([128, 1], F32, tag="den")
                    nc.vector.tensor_scalar_add(den, e2, 1.0)
                    rec = rsm.tile([128, 1], F32, tag="rec")
                    nc.vector.reciprocal(rec, den)
                    et = rsm.tile([128, N_EXP], F32, tag="et")
                    nc.scalar.activation(et, t, mybir.ActivationFunctionType.Exp)
                    ge = rsm.tile([128, N_EXP], F32, tag="ge")
                    nc.vector.tensor_scalar(
                        out=ge, in0=t, scalar1=m2, scalar2=None,
                        op0=mybir.AluOpType.is_ge,
                    )
                    w = rsm.tile([128, N_EXP], F32, tag="w")
                    nc.vector.tensor_tensor(
                        out=w, in0=et, in1=ge, op=mybir.AluOpType.mult
                    )
                    comb = rsm.tile([128, N_EXP], F32, tag="comb",
                                    name=f"comb_{b}_{c}", bufs=8)
                    nc.vector.tensor_scalar_mul(comb, w, rec)
                    combs[b, c] = comb

            for b in range(N_BLKS):
                bsl = slice(b * BLK, (b + 1) * BLK)
                combt_sb = rbig.tile([N_EXP, BLK], BF16, tag="ct", name=f"ct_{b}")
                for c in range(BLK // 128):
                    csl = slice(c * 128, (c + 1) * 128)
                    pc = ps.tile([128, 512], F32, tag="ps", name=f"pc_{b}_{c}")
                    nc.tensor.transpose(
                        out=pc[:N_EXP, :128], in_=combs[b, c], identity=ident_sb
                    )
                    nc.vector.tensor_copy(out=combt_sb[:, csl], in_=pc[:N_EXP, :128])

                # expand combine across the 16 ranks of each expert:
                # combine_expT[j, n] = combT[j//16, n]  via  emat.T @ combT
                pce = ps.tile([128, 512], F32, tag="ps", name=f"pce_{b}")
                mm(pce, emat_sb, combt_sb, start=True, stop=True)
                hsb = rbig.tile([128, BLK], F32, tag="hs", name=f"hs_{b}")
                nc.vector.tensor_copy(out=hsb, in_=phs[b])
                nc.vector.tensor_tensor(
                    out=hwt_sb[:, bsl], in0=hsb, in1=pce, op=mybir.AluOpType.mult
                )

            # ---- phase 2: LoRA up-projection + base GEMM + bias ----
            for o in range(O_TILES):
                osl = slice(o * 512, (o + 1) * 512)
                accs = [
                    ps.tile([128, 512], F32, tag="ps", name=f"acc_{o}_{n}")
                    for n in range(N_TILES)
                ]
                # open each accumulator with the expert contribution: its PSUM
                # slot wait (DVE release) coalesces with the hwt DVE wait.
                for n in range(N_TILES):
                    mm(accs[n], hwt_sb[:, n * 128:(n + 1) * 128],
                       bft_sb[:, osl], start=True, stop=False)
                for k in range(K_TILES):
                    wtt = wt_pool.tile([128, 512], BF16, tag="wt",
                                       name=f"wt_{o}_{k}")
                    nc.scalar.dma_start(
                        out=wtt, in_=wt[k * 128:(k + 1) * 128, osl]
                    )
                    for n in range(N_TILES):
                        mm(accs[n], xts[k][:, n * 128:(n + 1) * 128], wtt,
                           start=False, stop=(k == K_TILES - 1))
                for n in range(N_TILES):
                    osb = out_pool.tile([128, 512], F32, tag="ob",
                                        name=f"ob_{o}_{n}")
                    nc.vector.tensor_tensor(
                        out=osb, in0=accs[n], in1=bias_sb[:, osl],
                        op=mybir.AluOpType.add,
                    )
                    nc.sync.dma_start(
                        out=out_ap[n * 128:(n + 1) * 128, osl], in_=osb
                    )
                    # WAR closer: makes the DVE (not the outbound DMA queue)
                    # the releaser of this staging slot, so the next tile's
                    # bias-add needs no cross-queue DMA wait.
                    nc.vector.memset(osb[0:1, 0:1], 0.0)

    if finalize:
        nc.finalize()
    _CACHE[key] = nc
    return nc


def _prep_inputs(x, W_base, b_base, W_router, A, B):
    """Shard + lay out inputs for the 8 cores. Returns list of in_maps."""
    import ml_dtypes
    bf16 = ml_dtypes.bfloat16
    x_flat = np.ascontiguousarray(x, dtype=np.float32).reshape(-1, D_IN)
    wt = np.ascontiguousarray(W_base.T.astype(bf16))
    at = np.ascontiguousarray(
        A.astype(np.float32, copy=False)
        .transpose(2, 0, 1)
        .reshape(K_TILES, 128, J)
        .transpose(1, 0, 2)
        .astype(bf16)
    )
    wrt = (
        W_router.T.astype(np.float32, copy=False)
        .reshape(K_TILES, 128, N_EXP)
        .transpose(1, 0, 2)
    )
    bft = np.ascontiguousarray(
        (SCALING * B.astype(np.float32, copy=False).transpose(0, 2, 1)
         .reshape(J, D_OUT)).astype(bf16)
    )
    bvec = np.ascontiguousarray(b_base, dtype=np.float32)
    # packed bf16 residents: [:, :256] wrt, [:8, 256:384] emat
    pkb = np.zeros((128, 384), dtype=bf16)
    pkb[:, 0:256] = wrt.reshape(128, K_TILES * N_EXP).astype(bf16)
    pkb[0:N_EXP, 256:384] = np.repeat(
        np.eye(N_EXP, dtype=np.float32), R, axis=1
    ).astype(bf16)
    ident = np.eye(128, dtype=np.float32)

    in_maps = []
    for c in range(N_CORES):
        shard = x_flat[c * TOK:(c + 1) * TOK]          # [1024, 4096]
        xt = np.ascontiguousarray(shard.T.astype(bf16)).reshape(K_TILES, 128, TOK)
        in_maps.append({
            "xt": xt, "wt": wt, "at": at, "bft": bft, "bvec": bvec,
            "pkb": pkb, "ident": ident,
        })
    return in_maps


def _run(in_maps, trace=False, **kw):
    nc = _build_program()
    return run_bass_kernel_spmd(
        nc, in_maps, core_ids=list(range(N_CORES)), trace=trace, **kw
    )


def kernel(x, W_base, b_base, W_router, A, B):
    orig_shape = np.asarray(x).shape
    in_maps = _prep_inputs(x, W_base, b_base, W_router, A, B)
    res = _run(in_maps)
    shards = [res.results[c]["out"] for c in range(N_CORES)]
    out = np.concatenate(shards, axis=0)
    return out.reshape(*orig_shape[:-1], D_OUT).astype(np.float32, copy=False)


# revision 20
# speedup vs baseline: 1.4095x; 1.0188x over previous
"""MoE-LoRA layer (nn_MoELoRALayer) as a Bass/Tile kernel for 8 Trainium2 cores.

Computation (per token n):
    logits = x @ W_router.T                    # [N, 8]
    combine = renorm(top2(softmax(logits)))    # [N, 8]
    h       = x @ A_cat.T                      # [N, 128]   (8 experts x rank 16)
    hw      = h * combine_expanded             # [N, 128]
    out     = x @ W_base.T + b + 2.0 * hw @ B_cat.T

Sharding: data-parallel over tokens (1024 per core), weights replicated.
All heavy matmuls run as float32r (full-rate fp32 on the PE at N>=256),
accumulating in fp32 PSUM.

fp32r matmuls lower to an S3_LW uop with a single sync-wait slot, so the
program is arranged so every matmul depends on at most one new semaphore:
resident weights are "absorbed" onto the PE clock by tiny transposes at
program start, the router matmul precedes the A-path matmul per K-tile
(covering the xt tile's DMA), and each output tile's accumulation opens
with the LoRA up-projection (whose PSUM-slot wait coalesces with its DVE
data wait) before the W_base K-loop streams in.

Host-side layout prep (part of sharding):
    xt   [32, 128, 1024] = x_shard.T, K-tile major  (contraction d on partitions)
    wt   [4096, 4096]    = W_base.T
    at   [128, 32, 128]  = A.transpose(2,0,1) packed per K-tile (j = e*16+r)
    wrt  [128, 32, 8]    = W_router.T packed per K-tile
    bft  [128, 4096]     = 2.0 * B.transpose(0,2,1).reshape(128, 4096)
    emat [8, 128]        = expansion matrix (emat[e, j] = j//16 == e)
"""

import numpy as np

import concourse.bacc as bacc
import concourse.bass as bass
import concourse.mybir as mybir
import concourse.tile as tile
from concourse.bass_utils import run_bass_kernel_spmd

N_CORES = 8
D_IN = 4096
D_OUT = 4096
N_EXP = 8
R = 16
J = N_EXP * R           # 128
SCALING = 2.0
TOK = 1024              # tokens per core
K_TILES = D_IN // 128   # 32
N_TILES = TOK // 128    # 8
O_TILES = D_OUT // 512  # 8
BLK = 512               # token block for phase 1
N_BLKS = TOK // BLK     # 2

F32 = mybir.dt.float32
BF16 = mybir.dt.bfloat16

_CACHE = {}


def _build_program(finalize=True):
    key = ("nc", finalize)
    if key in _CACHE:
        return _CACHE[key]

    nc = bacc.Bacc(trn_type="TRN2")

    xt_d = nc.dram_tensor("xt", [K_TILES, 128, TOK], BF16, kind="ExternalInput")
    wt_d = nc.dram_tensor("wt", [D_IN, D_OUT], BF16, kind="ExternalInput")
    at_d = nc.dram_tensor("at", [128, K_TILES, J], BF16, kind="ExternalInput")
    bft_d = nc.dram_tensor("bft", [J, D_OUT], BF16, kind="ExternalInput")
    bvec_d = nc.dram_tensor("bvec", [D_OUT], F32, kind="ExternalInput")
    pkb_d = nc.dram_tensor("pkb", [128, 384], BF16, kind="ExternalInput")
    id_d = nc.dram_tensor("ident", [128, 128], F32, kind="ExternalInput")
    out_d = nc.dram_tensor("out", [TOK, D_OUT], F32, kind="ExternalOutput")

    xt = xt_d[:]
    wt = wt_d[:]
    out_ap = out_d[:]

    mm = nc.tensor.matmul

    with tile.TileContext(nc) as tc:
        with (
            tc.tile_pool(name="xt_pool", bufs=K_TILES) as xt_pool,
            tc.tile_pool(name="res", bufs=1) as res,
            tc.tile_pool(name="wt_pool", bufs=8) as wt_pool,
            tc.tile_pool(name="out_pool", bufs=6) as out_pool,
            tc.tile_pool(name="rsm", bufs=2) as rsm,
            tc.tile_pool(name="rbig", bufs=2) as rbig,
            tc.tile_pool(name="ps", bufs=8, space="PSUM") as ps,
        ):
            # ---- resident loads: small tensors first so phase 1 can start
            # while the xt stream is still arriving ----
            pkr = res.tile([128, 384], BF16)
            nc.sync.dma_start(out=pkr, in_=pkb_d[:])
            pkf = res.tile([128, 128], F32)
            nc.sync.dma_start(out=pkf, in_=id_d[:])
            at_sb = res.tile([128, K_TILES, J], BF16)
            nc.sync.dma_start(out=at_sb, in_=at_d[:])
            bft_sb = res.tile([J, D_OUT], BF16)
            nc.sync.dma_start(out=bft_sb, in_=bft_d[:])
            wrt_sb = pkr[:, 0:256].rearrange("p (k e) -> p k e", e=N_EXP)
            emat_sb = pkr[0:N_EXP, 256:384]
            ident_sb = pkf
            hwt_sb = res.tile([J, TOK], BF16)

            xts = []
            for k in range(K_TILES):
                t = xt_pool.tile([128, TOK], BF16, tag="xt", name=f"xt_{k}")
                nc.sync.dma_start(out=t, in_=xt[k])
                xts.append(t)

            bias_sb = res.tile([128, D_OUT], F32)
            nc.gpsimd.dma_start(
                out=bias_sb, in_=bvec_d[:].partition_broadcast(128)
            )

            # bias arrives on a SWDGE queue; observe it on the DVE clock once.
            btch = rsm.tile([1, 1], F32, tag="btch")
            nc.vector.tensor_copy(out=btch, in_=bias_sb[0:1, 0:1])

            # ---- phase 1: router + LoRA down-projection ----
            # Both blocks' K-loops run back-to-back on the PE (they consume the
            # incoming xt stream in order); the serial routing tails follow and
            # overlap with the start of phase 2.
            prs, phs = [], []
            for b in range(N_BLKS):
                bsl = slice(b * BLK, (b + 1) * BLK)
                pr = ps.tile([128, 512], F32, tag="ps", name=f"pr_{b}")
                ph = ps.tile([128, 512], F32, tag="ps", name=f"ph_{b}")
                for k in range(K_TILES):
                    # router first: its wait covers xts[k] for the A-path mm
                    mm(pr[:N_EXP, :], wrt_sb[:, k, :], xts[k][:, bsl],
                       start=(k == 0), stop=(k == K_TILES - 1))
                    mm(ph, at_sb[:, k, :], xts[k][:, bsl],
                       start=(k == 0), stop=(k == K_TILES - 1))
                prs.append(pr)
                phs.append(ph)

            logits = []
            for b in range(N_BLKS):
                logits_sb = rbig.tile([N_EXP, BLK], F32, tag="lg", name=f"lg_{b}")
                nc.vector.tensor_copy(out=logits_sb, in_=prs[b][:N_EXP, :])
                logits.append(logits_sb)

            # all forward transposes first: the PE never queues behind the
            # serial DVE/ACT routing chain (FIFO head-of-line blocking)
            ltoks = {}
            for b in range(N_BLKS):
                for c in range(BLK // 128):
                    csl = slice(c * 128, (c + 1) * 128)
                    pt = ps.tile([128, 512], F32, tag="ps", name=f"pt_{b}_{c}")
                    nc.tensor.transpose(
                        out=pt[:, :N_EXP],
                        in_=logits[b][:, csl],
                        identity=ident_sb[:N_EXP, :N_EXP],
                    )
                    ltok = rsm.tile([128, N_EXP], F32, tag="lt",
                                    name=f"lt_{b}_{c}", bufs=8)
                    nc.vector.tensor_copy(out=ltok, in_=pt[:, :N_EXP])
                    ltoks[b, c] = ltok

            # top-2 renormalized softmax weights, exact algebra:
            #   m1 = max_e l; t = l - m1; m2 = max_e (t | top1 -> -inf)
            #   combine_e = [t >= m2] * exp(t) / (1 + exp(m2))
            combs = {}
            for b in range(N_BLKS):
                for c in range(BLK // 128):
                    ltok = ltoks[b, c]
                    m1 = rsm.tile([128, 1], F32, tag="m1")
                    nc.vector.tensor_reduce(
                        m1, ltok, axis=mybir.AxisListType.X, op=mybir.AluOpType.max
                    )
                    t = rsm.tile([128, N_EXP], F32, tag="t")
                    nc.vector.tensor_scalar(
                        out=t, in0=ltok, scalar1=m1, scalar2=None,
                        op0=mybir.AluOpType.subtract,
                    )
                    eq = rsm.tile([128, N_EXP], F32, tag="eq")
                    nc.vector.tensor_scalar(
                        out=eq, in0=t, scalar1=0.0, scalar2=None,
                        op0=mybir.AluOpType.is_ge,
                    )
                    msk = rsm.tile([128, N_EXP], F32, tag="msk")
                    nc.vector.scalar_tensor_tensor(
                        out=msk, in0=eq, scalar=-1e30, in1=t,
                        op0=mybir.AluOpType.mult, op1=mybir.AluOpType.add,
                    )
                    m2 = rsm.tile([128, 1], F32, tag="m2")
                    nc.vector.tensor_reduce(
                        m2, msk, axis=mybir.AxisListType.X, op=mybir.AluOpType.max
                    )
                    e2 = rsm.tile([128, 1], F32, tag="e2")
                    nc.scalar.activation(e2, m2, mybir.ActivationFunctionType.Exp)
                    den = rsm.tile([128, 1], F32, tag="den")
                    nc.vector.tensor_scalar_add(den, e2, 1.0)
                    rec = rsm.tile([128, 1], F32, tag="rec")
                    nc.vector.reciprocal(rec, den)
                    et = rsm.tile([128, N_EXP], F32, tag="et")
                    nc.scalar.activation(et, t, mybir.ActivationFunctionType.Exp)
                    ge = rsm.tile([128, N_EXP], F32, tag="ge")
                    nc.vector.tensor_scalar(
                        out=ge, in0=t, scalar1=m2, scalar2=None,
                        op0=mybir.AluOpType.is_ge,
                    )
                    w = rsm.tile([128, N_EXP], F32, tag="w")
                    nc.vector.tensor_tensor(
                        out=w, in0=et, in1=ge, op=mybir.AluOpType.mult
                    )
                    comb = rsm.tile([128, N_EXP], F32, tag="comb",
                                    name=f"comb_{b}_{c}", bufs=8)
                    nc.vector.tensor_scalar_mul(comb, w, rec)
                    combs[b, c] = comb

            for b in range(N_BLKS):
                bsl = slice(b * BLK, (b + 1) * BLK)
                combt_sb = rbig.tile([N_EXP, BLK], BF16, tag="ct", name=f"ct_{b}")
                for c in range(BLK // 128):
                    csl = slice(c * 128, (c + 1) * 128)
                    pc = ps.tile([128, 512], F32, tag="ps", name=f"pc_{b}_{c}")
                    nc.tensor.transpose(
                        out=pc[:N_EXP, :128], in_=combs[b, c], identity=ident_sb
                    )
                    nc.vector.tensor_copy(out=combt_sb[:, csl], in_=pc[:N_EXP, :128])

                # expand combine across the 16 ranks of each expert:
                # combine_expT[j, n] = combT[j//16, n]  via  emat.T @ combT
                pce = ps.tile([128, 512], F32, tag="ps", name=f"pce_{b}")
                mm(pce, emat_sb, combt_sb, start=True, stop=True)
                hsb = rbig.tile([128, BLK], F32, tag="hs", name=f"hs_{b}")
                nc.vector.tensor_copy(out=hsb, in_=phs[b])
                nc.vector.tensor_tensor(
                    out=hwt_sb[:, bsl], in0=hsb, in1=pce, op=mybir.AluOpType.mult
                )

            # ---- phase 2: LoRA up-projection + base GEMM + bias ----
            for o in range(O_TILES):
                osl = slice(o * 512, (o + 1) * 512)
                accs = [
                    ps.tile([128, 512], F32, tag="ps", name=f"acc_{o}_{n}")
                    for n in range(N_TILES)
                ]
                # open each accumulator with the expert contribution: its PSUM
                # slot wait (DVE release) coalesces with the hwt DVE wait.
                for n in range(N_TILES):
                    mm(accs[n], hwt_sb[:, n * 128:(n + 1) * 128],
                       bft_sb[:, osl], start=True, stop=False)
                for k in range(K_TILES):
                    wtt = wt_pool.tile([128, 512], BF16, tag="wt",
                                       name=f"wt_{o}_{k}")
                    nc.scalar.dma_start(
                        out=wtt, in_=wt[k * 128:(k + 1) * 128, osl]
                    )
                    for n in range(N_TILES):
                        mm(accs[n], xts[k][:, n * 128:(n + 1) * 128], wtt,
                           start=False, stop=(k == K_TILES - 1))
                for n in range(N_TILES):
                    osb = out_pool.tile([128, 512], F32, tag="ob",
                                        name=f"ob_{o}_{n}")
                    nc.vector.tensor_tensor(
                        out=osb, in0=accs[n], in1=bias_sb[:, osl],
                        op=mybir.AluOpType.add,
                    )
                    nc.sync.dma_start(
                        out=out_ap[n * 128:(n + 1) * 128, osl], in_=osb
                    )
                    if o < O_TILES - 1:
                        # WAR closer: makes the DVE (not the outbound DMA
                        # queue) the releaser of this staging slot, so the
                        # next tile's bias-add needs no cross-queue DMA wait.
                        nc.vector.memset(osb[0:1, 0:1], 0.0)

    if finalize:
        nc.finalize()
    _CACHE[key] = nc
    return nc


def _prep_inputs(x, W_base, b_base, W_router, A, B):
    """Shard + lay out inputs for the 8 cores. Returns list of in_maps."""
    import ml_dtypes
    bf16 = ml_dtypes.bfloat16
    x_flat = np.ascontiguousarray(x, dtype=np.float32).reshape(-1, D_IN)
    wt = np.ascontiguousarray(W_base.T.astype(bf16))
    at = np.ascontiguousarray(
        A.astype(np.float32, copy=False)
        .transpose(2, 0, 1)
        .reshape(K_TILES, 128, J)
        .transpose(1, 0, 2)
        .astype(bf16)
    )
    wrt = (
        W_router.T.astype(np.float32, copy=False)
        .reshape(K_TILES, 128, N_EXP)
        .transpose(1, 0, 2)
    )
    bft = np.ascontiguousarray(
        (SCALING * B.astype(np.float32, copy=False).transpose(0, 2, 1)
         .reshape(J, D_OUT)).astype(bf16)
    )
    bvec = np.ascontiguousarray(b_base, dtype=np.float32)
    # packed bf16 residents: [:, :256] wrt, [:8, 256:384] emat
    pkb = np.zeros((128, 384), dtype=bf16)
    pkb[:, 0:256] = wrt.reshape(128, K_TILES * N_EXP).astype(bf16)
    pkb[0:N_EXP, 256:384] = np.repeat(
        np.eye(N_EXP, dtype=np.float32), R, axis=1
    ).astype(bf16)
    ident = np.eye(128, dtype=np.float32)

    in_maps = []
    for c in range(N_CORES):
        shard = x_flat[c * TOK:(c + 1) * TOK]          # [1024, 4096]
        xt = np.ascontiguousarray(shard.T.astype(bf16)).reshape(K_TILES, 128, TOK)
        in_maps.append({
            "xt": xt, "wt": wt, "at": at, "bft": bft, "bvec": bvec,
            "pkb": pkb, "ident": ident,
        })
    return in_maps


def _run(in_maps, trace=False, **kw):
    nc = _build_program()
    return run_bass_kernel_spmd(
        nc, in_maps, core_ids=list(range(N_CORES)), trace=trace, **kw
    )


def kernel(x, W_base, b_base, W_router, A, B):
    orig_shape = np.asarray(x).shape
    in_maps = _prep_inputs(x, W_base, b_base, W_router, A, B)
    res = _run(in_maps)
    shards = [res.results[c]["out"] for c in range(N_CORES)]
    out = np.concatenate(shards, axis=0)
    return out.reshape(*orig_shape[:-1], D_OUT).astype(np.float32, copy=False)
